# revision 7
# baseline (speedup 1.0000x reference)
"""Distributed 3-layer GAT encoder on 8 TRN2 NeuronCores (Bass/Tile).

Strategy (graph partition by dst):
  - Core c owns dst nodes [2500c, 2500c+2500), padded to 2560 = 20 x 128.
  - Per layer, the full transposed feature table xT [in_c, 20480] fp32
    lives in SBUF (built from the AllGathered previous layer outputs).
  - Per 128-dst block (edges dst-sorted in tiles of 128):
      gpsimd.ap_gather pulls xT columns by src -> xg [in_c, e] (SBUF);
      PE matmul lhsT=xg rhs=[W|was] -> per-edge [h | alpha_src] in PSUM
      (edge-partitioned directly, no transposes);
      alpha_dst accumulated into the same PSUM cols via a small matmul
      with lhsT=indT (host-precomputed indicator, DMA-streamed);
      es=leaky_relu on DVE, p=exp on ACT, pex=h*p on DVE (broadcast AP);
      numerator+denominator accumulated in PSUM via matmul lhsT=ind
      (also host-precomputed + streamed).
  - Flush: per-head normalize, mean over heads, bias, relu -> PE
    transpose -> fp32 AllGather -> next layer xT reload.
"""
import numpy as np

N = 20000
NCORES = 8
NPC = 2500
NPAD = 2560
NBLK = 20
NTOT = NCORES * NPAD  # 20480
P = 128

LAST_RESULT = None


# ----------------------------------------------------------------- host prep
def _wrap16(idx, ncols):
    n = len(idx)
    w = np.zeros((P, ncols), dtype=np.int16)
    cols = (n + 15) // 16
    assert cols <= ncols
    buf = np.zeros((16, cols), dtype=np.int16)
    buf[np.arange(n) % 16, np.arange(n) // 16] = idx
    for g in range(8):
        w[16 * g:16 * g + 16, :cols] = buf
    return w


def _preprocess(edge_index):
    src = np.asarray(edge_index[0], dtype=np.int64)
    dst = np.asarray(edge_index[1], dtype=np.int64)
    loop = np.arange(N, dtype=np.int64)
    src = np.concatenate([src, loop])
    dst = np.concatenate([dst, loop])

    own_s = src // NPC
    src_p = own_s * NPAD + (src - own_s * NPC)
    own = dst // NPC
    dst_loc = dst - own * NPC

    order = np.lexsort((dst_loc, own))
    src_p, dst_loc, own = src_p[order], dst_loc[order], own[order]
    blk = dst_loc // P
    counts = np.zeros((NCORES, NBLK), dtype=np.int64)
    for c in range(NCORES):
        for b in range(NBLK):
            counts[c, b] = np.sum((own == c) & (blk == b))
    T = np.maximum(1, np.ceil(counts.max(axis=0) / P).astype(np.int64))
    Ttot = int(T.sum())

    wrap_src = np.zeros((NCORES, P, Ttot * 8), dtype=np.int16)
    indh = np.zeros((NCORES, P, Ttot * P), dtype=np.float16)
    indTh = np.zeros((NCORES, P, Ttot * P), dtype=np.float16)
    off8 = np.zeros(NBLK + 1, dtype=np.int64)
    offT = np.zeros(NBLK + 1, dtype=np.int64)
    for b in range(NBLK):
        off8[b + 1] = off8[b] + T[b] * 8
        offT[b + 1] = offT[b] + T[b]
    jj = np.arange(P)
    for c in range(NCORES):
        m_c = own == c
        for b in range(NBLK):
            m = m_c & (blk == b)
            cnt = int(counts[c, b])
            Tb = int(T[b])
            nb = Tb * P
            isrc = np.zeros(nb, dtype=np.int64)
            isrc[:cnt] = src_p[m]
            dl = np.full(nb, -1, dtype=np.int64)
            dl[:cnt] = dst_loc[m] - b * P
            wrap_src[c, :, off8[b]:off8[b + 1]] = _wrap16(isrc, Tb * 8)
            dlm = dl.reshape(Tb, P).T  # [e, t]
            s, e = offT[b] * P, offT[b + 1] * P
            # ind[e, t*128+j] = (dl[t*128+e] == j)
            indh[c, :, s:e] = (
                dlm[:, :, None] == jj[None, None, :]
            ).reshape(P, Tb * P).astype(np.float16)
            # indT[j, t*128+e] = (dl[t*128+e] == j)
            indTh[c, :, s:e] = (
                jj[:, None, None] == dlm.T[None, :, :]
            ).reshape(P, Tb * P).astype(np.float16)
    return T, off8, offT, wrap_src, indh, indTh


# ------------------------------------------------------------- build program
def _build(T, off8, offT, do_compile=True):
    from concourse import bass, bacc, mybir, tile

    f16 = mybir.dt.float16
    f32 = mybir.dt.float32
    i16 = mybir.dt.int16
    AF = mybir.ActivationFunctionType
    OP = mybir.AluOpType

    Ttot = int(T.sum())
    Tmax = int(T.max())
    NW = Ttot * 8
    NVALID_LAST = NPC - (NBLK - 1) * P  # 68

    nc = bacc.Bacc("TRN2", target_bir_lowering=False, debug=False,
                   num_devices=NCORES)

    # inputs
    xT1f = nc.dram_tensor("xT1f", [P, NTOT], f32, kind="ExternalInput")
    xloc1 = nc.dram_tensor("xloc1", [P, NPAD], f32, kind="ExternalInput")
    iwsrc = nc.dram_tensor("iwsrc", [P, NW], i16, kind="ExternalInput")
    indh = nc.dram_tensor("indh", [P, Ttot * P], f16, kind="ExternalInput")
    indTh = nc.dram_tensor("indTh", [P, Ttot * P], f16, kind="ExternalInput")
    c100 = nc.dram_tensor("c100", [P, 32], f32, kind="ExternalInput")
    c1em8 = nc.dram_tensor("c1em8", [P, 32], f32, kind="ExternalInput")
    identf = nc.dram_tensor("identf", [P, P], f32, kind="ExternalInput")
    w1c = nc.dram_tensor("w1c", [128, 260], f32, kind="ExternalInput")
    w2c = nc.dram_tensor("w2c", [64, 260], f32, kind="ExternalInput")
    w3c = nc.dram_tensor("w3c", [64, 33], f32, kind="ExternalInput")
    wad1 = nc.dram_tensor("wad1", [128, 4], f32, kind="ExternalInput")
    wad2 = nc.dram_tensor("wad2", [64, 4], f32, kind="ExternalInput")
    wad3 = nc.dram_tensor("wad3", [64, 1], f32, kind="ExternalInput")
    b1r = nc.dram_tensor("b1r", [P, 64], f32, kind="ExternalInput")
    b2r = nc.dram_tensor("b2r", [P, 64], f32, kind="ExternalInput")
    b3r = nc.dram_tensor("b3r", [P, 32], f32, kind="ExternalInput")
    bmr = nc.dram_tensor("bmr", [P, 32], f32, kind="ExternalInput")
    bvr = nc.dram_tensor("bvr", [P, 32], f32, kind="ExternalInput")
    wm = nc.dram_tensor("wm", [32, 32], f32, kind="ExternalInput")
    wv = nc.dram_tensor("wv", [32, 32], f32, kind="ExternalInput")

    # outputs
    z_out = nc.dram_tensor("z", [NPC, 32], f32, kind="ExternalOutput")
    zm_out = nc.dram_tensor("zmean", [NPC, 32], f32, kind="ExternalOutput")
    zv_out = nc.dram_tensor("zvar", [NPC, 32], f32, kind="ExternalOutput")

    with tile.TileContext(nc) as tc:
        with (
            tc.tile_pool(name="const", bufs=1) as cpool,
            tc.tile_pool(name="xtp", bufs=1) as xtpool,
            tc.tile_pool(name="xg", bufs=3) as xgp,
            tc.tile_pool(name="indp", bufs=3) as indp,
            tc.tile_pool(name="indtp", bufs=3) as indtp,
            tc.tile_pool(name="pex", bufs=3) as pexp,
            tc.tile_pool(name="sb", bufs=3) as sb,
            tc.tile_pool(name="pshp", bufs=4, space="PSUM") as pshp,
            tc.tile_pool(name="psagg", bufs=2, space="PSUM") as psagg,
            tc.tile_pool(name="pssm", bufs=1, space="PSUM") as pssm,
            tc.tile_pool(name="dram", bufs=1, space="DRAM") as dram,
        ):
            x2T_loc = dram.tile([64, NPAD], f32)
            x3T_loc = dram.tile([64, NPAD], f32)
            x2T_full = dram.tile([NCORES, 64, NPAD], f32)
            x3T_full = dram.tile([NCORES, 64, NPAD], f32)

            def ld(shape, dt, src):
                t = cpool.tile(shape, dt, tag="c_" + src.name)
                nc.sync.dma_start(out=t[:], in_=src[:, :])
                return t

            idf_sb = ld([P, P], f32, identf)
            w1c_sb = ld([128, 260], f32, w1c)
            w2c_sb = ld([64, 260], f32, w2c)
            w3c_sb = ld([64, 33], f32, w3c)
            wad1_sb = ld([128, 4], f32, wad1)
            wad2_sb = ld([64, 4], f32, wad2)
            wad3_sb = ld([64, 1], f32, wad3)
            b1r_sb = ld([P, 64], f32, b1r)
            b2r_sb = ld([P, 64], f32, b2r)
            b3r_sb = ld([P, 32], f32, b3r)
            bmr_sb = ld([P, 32], f32, bmr)
            bvr_sb = ld([P, 32], f32, bvr)
            wm_sb = ld([32, 32], f32, wm)
            wv_sb = ld([32, 32], f32, wv)
            c100_sb = ld([P, 32], f32, c100)
            c1em8_sb = ld([P, 32], f32, c1em8)
            iwsrc_sb = ld([P, NW], i16, iwsrc)
            xloc1_sb = ld([P, NPAD], f32, xloc1)

            xt_sb = xtpool.tile([P, NTOT], f32, tag="xt")
            ad1 = cpool.tile([P, NBLK, 4], f16, tag="ad1")
            ad2 = cpool.tile([P, NBLK, 4], f16, tag="ad2")
            ad3 = cpool.tile([P, NBLK, 1], f16, tag="ad3")

            # ---------------- alpha_dst local table --------------------
            def write_adloc(xt_blk, wad_sb, in_c, H, adt, b):
                pad = pssm.tile([P, 4], f32, space="PSUM", tag="fl")
                nc.tensor.matmul(out=pad[:, :H], lhsT=xt_blk,
                                 rhs=wad_sb[:in_c, :H], start=True, stop=True)
                nc.scalar.activation(adt[:, b, :], pad[:, :H], AF.Copy)

            # ---------------- edge phase -------------------------------
            CH = 4  # pipeline chunk (== pshp bufs)

            def edge_layer(in_c, wc_sb, adt, H, C, flush):
                HC = H * C
                WA = HC + H  # agg rhs width: [h*p | p]
                for b in range(NBLK):
                    Tb = int(T[b])
                    ni = Tb * P
                    s, e = int(offT[b]) * P, int(offT[b + 1]) * P
                    xg = xgp.tile([P, Tmax * P], f32, tag="xg")
                    nc.gpsimd.ap_gather(
                        out_ap=xg[0:in_c, 0:ni, None],
                        in_ap=xt_sb[0:in_c, :, None],
                        idxs_ap=iwsrc_sb[0:in_c,
                                         int(off8[b]):int(off8[b]) + Tb * 8],
                        channels=in_c, num_elems=NTOT, d=1, num_idxs=ni)
                    ind = indp.tile([P, Tmax, P], f16, tag="ind")
                    nc.sync.dma_start(
                        out=ind[:, 0:Tb, :].rearrange("p t q -> p (t q)"),
                        in_=indh[:, s:e])
                    indT = indtp.tile([P, Tmax, P], f16, tag="indT")
                    nc.scalar.dma_start(
                        out=indT[:, 0:Tb, :].rearrange("p t q -> p (t q)"),
                        in_=indTh[:, s:e])
                    pex = pexp.tile([P, Tmax, 264], f16, tag="pex")
                    pa = psagg.tile([P, 260], f32, space="PSUM", tag="agg")
                    for t0 in range(0, Tb, CH):
                        t1 = min(t0 + CH, Tb)
                        hps_l = []
                        for t in range(t0, t1):
                            hps = pshp.tile([P, 264], f32, space="PSUM",
                                            tag="hps")
                            hps_l.append(hps)
                            nc.tensor.matmul(
                                out=hps[:, 0:WA],
                                lhsT=xg[0:in_c, t * P:(t + 1) * P],
                                rhs=wc_sb[0:in_c, 0:WA],
                                start=True, stop=False)
                            nc.tensor.matmul(
                                out=hps[:, HC:WA], lhsT=indT[:, t, :],
                                rhs=adt[:, b, :], start=False, stop=True)
                            # es scratch -> pex[:, t, WA:WA+H]
                            nc.vector.tensor_scalar_mul(
                                out=pex[:, t, WA:WA + H],
                                in0=hps[:, HC:WA], scalar1=0.2)
                            nc.vector.tensor_max(
                                out=pex[:, t, WA:WA + H],
                                in0=pex[:, t, WA:WA + H],
                                in1=hps[:, HC:WA])
                        # p = exp(es), one ACT op per chunk
                        nc.scalar.activation(
                            pex[:, t0:t1, HC:WA],
                            pex[:, t0:t1, WA:WA + H], AF.Exp)
                        for t in range(t0, t1):
                            nc.vector.tensor_mul(
                                out=pex[:, t, 0:HC]
                                .rearrange("p (h c) -> p h c", h=H),
                                in0=hps_l[t - t0][:, 0:HC]
                                .rearrange("p (h c) -> p h c", h=H),
                                in1=pex[:, t, HC:WA, None]
                                .to_broadcast([P, H, C]))
                            nc.tensor.matmul(
                                out=pa[:, 0:WA], lhsT=ind[:, t, :],
                                rhs=pex[:, t, 0:WA],
                                start=(t == 0), stop=(t == Tb - 1))
                    flush(b, pa)

            # ---------------- flush ------------------------------------
            def flush_12(b, pa, H, C, brep_sb, xT_loc_dram, wadn_sb, adn,
                         Hn):
                HC = H * C
                inv = sb.tile([P, H], f32, tag="inv")
                nc.vector.tensor_scalar_add(out=inv[:], in0=pa[:, HC:HC + H],
                                            scalar1=1e-16)
                nc.vector.reciprocal(out=inv[:], in_=inv[:])
                nrm = sb.tile([P, HC], f32, tag="nrm")
                nc.vector.tensor_mul(
                    out=nrm[:].rearrange("p (h c) -> p h c", h=H),
                    in0=pa[:, 0:HC].rearrange("p (h c) -> p h c", h=H),
                    in1=inv[:, :, None].to_broadcast([P, H, C]))
                m = sb.tile([P, C], f32, tag="mean")
                nc.vector.tensor_reduce(
                    out=m[:], in_=nrm[:].rearrange("p (h c) -> p c h", h=H),
                    axis=mybir.AxisListType.X, op=OP.add)
                nc.vector.scalar_tensor_tensor(
                    out=m[:], in0=m[:], scalar=1.0 / H, in1=brep_sb[:, :C],
                    op0=OP.mult, op1=OP.add)
                xf = sb.tile([P, C], f32, tag="xf")
                nc.scalar.activation(xf[:], m[:], AF.Relu)
                pt = pssm.tile([C, P], f32, space="PSUM", tag="fl")
                nc.tensor.transpose(out=pt[:], in_=xf[:], identity=idf_sb[:])
                xts = sb.tile([C, P], f32, tag="xts")
                nc.scalar.activation(xts[:], pt[:], AF.Copy)
                nc.sync.dma_start(out=xT_loc_dram[:, b * P:(b + 1) * P],
                                  in_=xts[:])
                write_adloc(xts[:], wadn_sb, C, Hn, adn, b)

            def flush_3(b, pa):
                nvalid = NVALID_LAST if b == NBLK - 1 else P
                inv = sb.tile([P, 1], f32, tag="inv")
                nc.vector.tensor_scalar_add(out=inv[:], in0=pa[:, 32:33],
                                            scalar1=1e-16)
                nc.vector.reciprocal(out=inv[:], in_=inv[:])
                z = sb.tile([P, 32], f32, tag="zf")
                nc.vector.tensor_mul(out=z[:], in0=pa[:, 0:32],
                                     in1=inv[:, 0:1].to_broadcast([P, 32]))
                nc.vector.tensor_add(out=z[:], in0=z[:], in1=b3r_sb[:])
                nc.sync.dma_start(out=z_out[b * P:b * P + nvalid, :],
                                  in_=z[:nvalid, :])
                zt_ps = pssm.tile([32, P], f32, space="PSUM", tag="fl")
                nc.tensor.transpose(out=zt_ps[:], in_=z[:, :32],
                                    identity=idf_sb[:])
                zt = sb.tile([32, P], f32, tag="zt")
                nc.vector.tensor_copy(out=zt[:], in_=zt_ps[:])
                pm = pssm.tile([P, 32], f32, space="PSUM", tag="fl")
                nc.tensor.matmul(out=pm[:], lhsT=zt[:], rhs=wm_sb[:],
                                 start=True, stop=True)
                zm = sb.tile([P, 32], f32, tag="zm")
                nc.vector.tensor_add(out=zm[:], in0=pm[:], in1=bmr_sb[:])
                nc.sync.dma_start(out=zm_out[b * P:b * P + nvalid, :],
                                  in_=zm[:nvalid, :])
                pv = pssm.tile([P, 32], f32, space="PSUM", tag="fl")
                nc.tensor.matmul(out=pv[:], lhsT=zt[:], rhs=wv_sb[:],
                                 start=True, stop=True)
                zv = sb.tile([P, 32], f32, tag="zv")
                nc.vector.tensor_add(out=zv[:], in0=pv[:], in1=bvr_sb[:])
                nc.scalar.activation(zv[:], zv[:], AF.Exp)
                nc.vector.tensor_tensor(out=zv[:], in0=zv[:], in1=c100_sb[:],
                                        op=OP.min)
                nc.vector.tensor_tensor(out=zv[:], in0=zv[:], in1=c1em8_sb[:],
                                        op=OP.max)
                nc.sync.dma_start(out=zv_out[b * P:b * P + nvalid, :],
                                  in_=zv[:nvalid, :])

            # ================ the program ==============================
            nc.sync.dma_start(out=xt_sb[:], in_=xT1f[:, :])
            for b in range(NBLK):
                write_adloc(xloc1_sb[:, b * P:(b + 1) * P], wad1_sb, 128, 4,
                            ad1, b)
            edge_layer(128, w1c_sb, ad1, 4, 64,
                       lambda b, pa: flush_12(b, pa, 4, 64, b1r_sb, x2T_loc,
                                              wad2_sb, ad2, 4))
            nc.gpsimd.collective_compute(
                "AllGather", mybir.AluOpType.bypass,
                replica_groups=[list(range(NCORES))],
                ins=[x2T_loc[:]], outs=[x2T_full[:]])
            for c in range(NCORES):
                eng = nc.sync if c % 2 == 0 else nc.scalar
                eng.dma_start(out=xt_sb[0:64, c * NPAD:(c + 1) * NPAD],
                              in_=x2T_full[c])
            edge_layer(64, w2c_sb, ad2, 4, 64,
                       lambda b, pa: flush_12(b, pa, 4, 64, b2r_sb, x3T_loc,
                                              wad3_sb, ad3, 1))
            nc.gpsimd.collective_compute(
                "AllGather", mybir.AluOpType.bypass,
                replica_groups=[list(range(NCORES))],
                ins=[x3T_loc[:]], outs=[x3T_full[:]])
            for c in range(NCORES):
                eng = nc.sync if c % 2 == 0 else nc.scalar
                eng.dma_start(out=xt_sb[0:64, c * NPAD:(c + 1) * NPAD],
                              in_=x3T_full[c])
            edge_layer(64, w3c_sb, ad3, 1, 32, flush_3)

    if do_compile:
        nc.compile()
    return nc


def _make_in_maps(x, params, wrap_src, indh, indTh):
    x = np.asarray(x, dtype=np.float32)

    def comb(W, a_s):
        W = np.asarray(W, np.float32)
        a_s = np.asarray(a_s, np.float32)
        heads, c = a_s.shape
        Wr = W.reshape(W.shape[0], heads, c)
        was = np.einsum('ihc,hc->ih', Wr, a_s)
        return np.concatenate([W, was], axis=1).astype(np.float32)

    def wadf(W, a_d):
        W = np.asarray(W, np.float32)
        a_d = np.asarray(a_d, np.float32)
        heads, c = a_d.shape
        Wr = W.reshape(W.shape[0], heads, c)
        return np.einsum('ihc,hc->ih', Wr, a_d).astype(np.float32)

    xT1f = np.zeros((P, NTOT), dtype=np.float32)
    for c in range(NCORES):
        xs = x[c * NPC:(c + 1) * NPC]
        xT1f[:, c * NPAD:c * NPAD + NPC] = xs.T

    def rep(v, n=P):
        v = np.asarray(v, np.float32).reshape(1, -1)
        return np.repeat(v, n, axis=0).astype(np.float32)

    common = dict(
        xT1f=xT1f,
        c100=np.full((P, 32), 100.0, dtype=np.float32),
        c1em8=np.full((P, 32), 1e-8, dtype=np.float32),
        identf=np.eye(P, dtype=np.float32),
        w1c=comb(params['W1'], params['as1']),
        w2c=comb(params['W2'], params['as2']),
        w3c=comb(params['W3'], params['as3']),
        wad1=wadf(params['W1'], params['ad1']),
        wad2=wadf(params['W2'], params['ad2']),
        wad3=wadf(params['W3'], params['ad3']),
        b1r=rep(params['b1']), b2r=rep(params['b2']), b3r=rep(params['b3']),
        bmr=rep(params['bm']), bvr=rep(params['bv']),
        wm=np.asarray(params['Wm'], np.float32),
        wv=np.asarray(params['Wv'], np.float32),
    )
    in_maps = []
    for c in range(NCORES):
        m = dict(common)
        m.update(iwsrc=wrap_src[c], indh=indh[c], indTh=indTh[c],
                 xloc1=xT1f[:, c * NPAD:(c + 1) * NPAD].copy())
        in_maps.append(m)
    return in_maps


# ------------------------------------------------------------------ driver
def kernel(x, edge_index, W1, as1, ad1, b1, W2, as2, ad2, b2,
           W3, as3, ad3, b3, Wm, bm, Wv, bv):
    global LAST_RESULT
    import os
    from concourse.bass_utils import run_bass_kernel_spmd

    T, off8, offT, wrap_src, indh, indTh = _preprocess(
        np.asarray(edge_index))
    params = dict(W1=W1, as1=as1, ad1=ad1, b1=b1, W2=W2, as2=as2, ad2=ad2,
                  b2=b2, W3=W3, as3=as3, ad3=ad3, b3=b3, Wm=Wm, bm=bm,
                  Wv=Wv, bv=bv)
    in_maps = _make_in_maps(x, params, wrap_src, indh, indTh)

    nc = _build(T, off8, offT)
    res = run_bass_kernel_spmd(
        nc, in_maps, core_ids=list(range(NCORES)),
        trace=os.environ.get("BASS_TRACE", "") not in ("", "0"))
    LAST_RESULT = res

    z = np.concatenate([res.results[c]["z"] for c in range(NCORES)], axis=0)
    zm = np.concatenate([res.results[c]["zmean"] for c in range(NCORES)],
                        axis=0)
    zv = np.concatenate([res.results[c]["zvar"] for c in range(NCORES)],
                        axis=0)
    return zm, zv, z


# revision 8
# speedup vs baseline: 1.0064x; 1.0064x over previous
"""Distributed 3-layer GAT encoder on 8 TRN2 NeuronCores (Bass/Tile).

Strategy (graph partition by dst):
  - Core c owns dst nodes [2500c, 2500c+2500), padded to 2560 = 20 x 128.
  - Per layer, the full transposed feature table xT [in_c, 20480] fp32
    lives in SBUF (built from the AllGathered previous layer outputs).
  - Per 128-dst block (edges dst-sorted in tiles of 128):
      gpsimd.ap_gather pulls xT columns by src -> xg [in_c, e] (SBUF);
      PE matmul lhsT=xg rhs=[W|was] -> per-edge [h | alpha_src] in PSUM
      (edge-partitioned directly, no transposes);
      alpha_dst accumulated into the same PSUM cols via a small matmul
      with lhsT=indT (host-precomputed indicator, DMA-streamed);
      es=leaky_relu on DVE, p=exp on ACT, pex=h*p on DVE (broadcast AP);
      numerator+denominator accumulated in PSUM via matmul lhsT=ind
      (also host-precomputed + streamed).
  - Flush: per-head normalize, mean over heads, bias, relu -> PE
    transpose -> fp32 AllGather -> next layer xT reload.
"""
import numpy as np

N = 20000
NCORES = 8
NPC = 2500
NPAD = 2560
NBLK = 20
NTOT = NCORES * NPAD  # 20480
P = 128

LAST_RESULT = None


# ----------------------------------------------------------------- host prep
def _wrap16(idx, ncols):
    n = len(idx)
    w = np.zeros((P, ncols), dtype=np.int16)
    cols = (n + 15) // 16
    assert cols <= ncols
    buf = np.zeros((16, cols), dtype=np.int16)
    buf[np.arange(n) % 16, np.arange(n) // 16] = idx
    for g in range(8):
        w[16 * g:16 * g + 16, :cols] = buf
    return w


def _preprocess(edge_index):
    src = np.asarray(edge_index[0], dtype=np.int64)
    dst = np.asarray(edge_index[1], dtype=np.int64)
    loop = np.arange(N, dtype=np.int64)
    src = np.concatenate([src, loop])
    dst = np.concatenate([dst, loop])

    own_s = src // NPC
    src_p = own_s * NPAD + (src - own_s * NPC)
    own = dst // NPC
    dst_loc = dst - own * NPC

    order = np.lexsort((dst_loc, own))
    src_p, dst_loc, own = src_p[order], dst_loc[order], own[order]
    blk = dst_loc // P
    counts = np.zeros((NCORES, NBLK), dtype=np.int64)
    for c in range(NCORES):
        for b in range(NBLK):
            counts[c, b] = np.sum((own == c) & (blk == b))
    T = np.maximum(1, np.ceil(counts.max(axis=0) / P).astype(np.int64))
    Ttot = int(T.sum())

    wrap_src = np.zeros((NCORES, P, Ttot * 8), dtype=np.int16)
    indh = np.zeros((NCORES, P, Ttot * P), dtype=np.float16)
    indTh = np.zeros((NCORES, P, Ttot * P), dtype=np.float16)
    off8 = np.zeros(NBLK + 1, dtype=np.int64)
    offT = np.zeros(NBLK + 1, dtype=np.int64)
    for b in range(NBLK):
        off8[b + 1] = off8[b] + T[b] * 8
        offT[b + 1] = offT[b] + T[b]
    jj = np.arange(P)
    for c in range(NCORES):
        m_c = own == c
        for b in range(NBLK):
            m = m_c & (blk == b)
            cnt = int(counts[c, b])
            Tb = int(T[b])
            nb = Tb * P
            isrc = np.zeros(nb, dtype=np.int64)
            isrc[:cnt] = src_p[m]
            dl = np.full(nb, -1, dtype=np.int64)
            dl[:cnt] = dst_loc[m] - b * P
            wrap_src[c, :, off8[b]:off8[b + 1]] = _wrap16(isrc, Tb * 8)
            dlm = dl.reshape(Tb, P).T  # [e, t]
            s, e = offT[b] * P, offT[b + 1] * P
            # ind[e, t*128+j] = (dl[t*128+e] == j)
            indh[c, :, s:e] = (
                dlm[:, :, None] == jj[None, None, :]
            ).reshape(P, Tb * P).astype(np.float16)
            # indT[j, t*128+e] = (dl[t*128+e] == j)
            indTh[c, :, s:e] = (
                jj[:, None, None] == dlm.T[None, :, :]
            ).reshape(P, Tb * P).astype(np.float16)
    return T, off8, offT, wrap_src, indh, indTh


# ------------------------------------------------------------- build program
def _build(T, off8, offT, do_compile=True):
    from concourse import bass, bacc, mybir, tile

    f16 = mybir.dt.float16
    f32 = mybir.dt.float32
    i16 = mybir.dt.int16
    AF = mybir.ActivationFunctionType
    OP = mybir.AluOpType

    Ttot = int(T.sum())
    Tmax = int(T.max())
    NW = Ttot * 8
    NVALID_LAST = NPC - (NBLK - 1) * P  # 68

    nc = bacc.Bacc("TRN2", target_bir_lowering=False, debug=False,
                   num_devices=NCORES)

    # inputs
    xT1f = nc.dram_tensor("xT1f", [P, NTOT], f32, kind="ExternalInput")
    xloc1 = nc.dram_tensor("xloc1", [P, NPAD], f32, kind="ExternalInput")
    iwsrc = nc.dram_tensor("iwsrc", [P, NW], i16, kind="ExternalInput")
    indh = nc.dram_tensor("indh", [P, Ttot * P], f16, kind="ExternalInput")
    indTh = nc.dram_tensor("indTh", [P, Ttot * P], f16, kind="ExternalInput")
    c100 = nc.dram_tensor("c100", [P, 32], f32, kind="ExternalInput")
    c1em8 = nc.dram_tensor("c1em8", [P, 32], f32, kind="ExternalInput")
    identf = nc.dram_tensor("identf", [P, P], f32, kind="ExternalInput")
    w1c = nc.dram_tensor("w1c", [128, 260], f32, kind="ExternalInput")
    w2c = nc.dram_tensor("w2c", [128, 260], f32, kind="ExternalInput")
    w3c = nc.dram_tensor("w3c", [128, 33], f32, kind="ExternalInput")
    wad1 = nc.dram_tensor("wad1", [128, 4], f32, kind="ExternalInput")
    wad2 = nc.dram_tensor("wad2", [64, 4], f32, kind="ExternalInput")
    wad3 = nc.dram_tensor("wad3", [64, 1], f32, kind="ExternalInput")
    b1r = nc.dram_tensor("b1r", [P, 64], f32, kind="ExternalInput")
    b2r = nc.dram_tensor("b2r", [P, 64], f32, kind="ExternalInput")
    b3r = nc.dram_tensor("b3r", [P, 32], f32, kind="ExternalInput")
    bmr = nc.dram_tensor("bmr", [P, 32], f32, kind="ExternalInput")
    bvr = nc.dram_tensor("bvr", [P, 32], f32, kind="ExternalInput")
    wm = nc.dram_tensor("wm", [32, 32], f32, kind="ExternalInput")
    wv = nc.dram_tensor("wv", [32, 32], f32, kind="ExternalInput")

    # outputs
    z_out = nc.dram_tensor("z", [NPC, 32], f32, kind="ExternalOutput")
    zm_out = nc.dram_tensor("zmean", [NPC, 32], f32, kind="ExternalOutput")
    zv_out = nc.dram_tensor("zvar", [NPC, 32], f32, kind="ExternalOutput")

    with tile.TileContext(nc) as tc:
        with (
            tc.tile_pool(name="const", bufs=1) as cpool,
            tc.tile_pool(name="xtp", bufs=1) as xtpool,
            tc.tile_pool(name="xg", bufs=3) as xgp,
            tc.tile_pool(name="indp", bufs=3) as indp,
            tc.tile_pool(name="indtp", bufs=3) as indtp,
            tc.tile_pool(name="pex", bufs=3) as pexp,
            tc.tile_pool(name="sb", bufs=3) as sb,
            tc.tile_pool(name="pshp", bufs=2, space="PSUM") as pshp,
            tc.tile_pool(name="psagg", bufs=2, space="PSUM") as psagg,
            tc.tile_pool(name="pssm", bufs=1, space="PSUM") as pssm,
            tc.tile_pool(name="dram", bufs=1, space="DRAM") as dram,
        ):
            x2T_loc = dram.tile([64, NPAD], f32)
            x3T_loc = dram.tile([64, NPAD], f32)
            x2T_full = dram.tile([NCORES, 64, NPAD], f32)
            x3T_full = dram.tile([NCORES, 64, NPAD], f32)

            def ld(shape, dt, src):
                t = cpool.tile(shape, dt, tag="c_" + src.name)
                nc.sync.dma_start(out=t[:], in_=src[:, :])
                return t

            idf_sb = ld([P, P], f32, identf)
            w1c_sb = ld([128, 260], f32, w1c)
            w2c_sb = ld([128, 260], f32, w2c)
            w3c_sb = ld([128, 33], f32, w3c)
            wad1_sb = ld([128, 4], f32, wad1)
            wad2_sb = ld([64, 4], f32, wad2)
            wad3_sb = ld([64, 1], f32, wad3)
            b1r_sb = ld([P, 64], f32, b1r)
            b2r_sb = ld([P, 64], f32, b2r)
            b3r_sb = ld([P, 32], f32, b3r)
            bmr_sb = ld([P, 32], f32, bmr)
            bvr_sb = ld([P, 32], f32, bvr)
            wm_sb = ld([32, 32], f32, wm)
            wv_sb = ld([32, 32], f32, wv)
            c100_sb = ld([P, 32], f32, c100)
            c1em8_sb = ld([P, 32], f32, c1em8)
            iwsrc_sb = ld([P, NW], i16, iwsrc)
            xloc1_sb = ld([P, NPAD], f32, xloc1)

            xt_sb = xtpool.tile([P, NTOT], f32, tag="xt")
            ad1 = cpool.tile([P, NBLK, 4], f16, tag="ad1")
            ad2 = cpool.tile([P, NBLK, 4], f16, tag="ad2")
            ad3 = cpool.tile([P, NBLK, 1], f16, tag="ad3")

            # ---------------- alpha_dst local table --------------------
            def write_adloc(xt_blk, wad_sb, in_c, H, adt, b):
                pad = pssm.tile([P, 4], f32, space="PSUM", tag="fl")
                nc.tensor.matmul(out=pad[:, :H], lhsT=xt_blk,
                                 rhs=wad_sb[:in_c, :H], start=True, stop=True)
                nc.scalar.activation(adt[:, b, :], pad[:, :H], AF.Copy)

            # ---------------- edge phase -------------------------------
            CH = 2  # tiles per PSUM chunk (chunk tile = CH banks)

            def edge_layer(in_c, wc_sb, adt, H, C, flush):
                HC = H * C
                WA = HC + H  # agg rhs width: [h*p | p]
                for b in range(NBLK):
                    Tb = int(T[b])
                    ni = Tb * P
                    s, e = int(offT[b]) * P, int(offT[b + 1]) * P
                    xg = xgp.tile([P, Tmax * P], f32, tag="xg")
                    nc.gpsimd.ap_gather(
                        out_ap=xg[0:in_c, 0:ni, None],
                        in_ap=xt_sb[0:in_c, :, None],
                        idxs_ap=iwsrc_sb[0:in_c,
                                         int(off8[b]):int(off8[b]) + Tb * 8],
                        channels=in_c, num_elems=NTOT, d=1, num_idxs=ni)
                    ind = indp.tile([P, Tmax, P], f16, tag="ind")
                    nc.sync.dma_start(
                        out=ind[:, 0:Tb, :].rearrange("p t q -> p (t q)"),
                        in_=indh[:, s:e])
                    indT = indtp.tile([P, Tmax, P], f16, tag="indT")
                    nc.scalar.dma_start(
                        out=indT[:, 0:Tb, :].rearrange("p t q -> p (t q)"),
                        in_=indTh[:, s:e])
                    pex = pexp.tile([P, Tmax, 264], f16, tag="pex")
                    pa = psagg.tile([P, 260], f32, space="PSUM", tag="agg")
                    for t0 in range(0, Tb, CH):
                        t1 = min(t0 + CH, Tb)
                        ch = t1 - t0
                        hw = pshp.tile([P, CH, 512], f32, space="PSUM",
                                       tag="hps")
                        for t in range(t0, t1):
                            nc.tensor.matmul(
                                out=hw[:, t - t0, 0:WA],
                                lhsT=xg[0:P, t * P:(t + 1) * P],
                                rhs=wc_sb[0:P, 0:WA],
                                start=True, stop=False)
                            nc.tensor.matmul(
                                out=hw[:, t - t0, HC:WA],
                                lhsT=indT[:, t, :],
                                rhs=adt[:, b, :], start=False, stop=True)
                        # es = leaky(as+ad) -> pex[:, t, WA:WA+H] (2 DVE ops)
                        nc.vector.tensor_scalar_mul(
                            out=pex[:, t0:t1, WA:WA + H],
                            in0=hw[:, 0:ch, HC:WA], scalar1=0.2)
                        nc.vector.tensor_max(
                            out=pex[:, t0:t1, WA:WA + H],
                            in0=pex[:, t0:t1, WA:WA + H],
                            in1=hw[:, 0:ch, HC:WA])
                        # p = exp(es) -> pex[:, t, HC:WA] (1 ACT op)
                        nc.scalar.activation(
                            pex[:, t0:t1, HC:WA],
                            pex[:, t0:t1, WA:WA + H], AF.Exp)
                        # pex = h * p (1 DVE op, broadcast in1)
                        nc.vector.tensor_mul(
                            out=pex[:, t0:t1, 0:HC]
                            .rearrange("p t (h c) -> p t h c", h=H),
                            in0=hw[:, 0:ch, 0:HC]
                            .rearrange("p t (h c) -> p t h c", h=H),
                            in1=pex[:, t0:t1, HC:WA, None]
                            .to_broadcast([P, ch, H, C]))
                        for t in range(t0, t1):
                            nc.tensor.matmul(
                                out=pa[:, 0:WA], lhsT=ind[:, t, :],
                                rhs=pex[:, t, 0:WA],
                                start=(t == 0), stop=(t == Tb - 1))
                    flush(b, pa)

            # ---------------- flush ------------------------------------
            def flush_12(b, pa, H, C, brep_sb, xT_loc_dram, wadn_sb, adn,
                         Hn):
                HC = H * C
                inv = sb.tile([P, H], f32, tag="inv")
                nc.vector.tensor_scalar_add(out=inv[:], in0=pa[:, HC:HC + H],
                                            scalar1=1e-16)
                nc.vector.reciprocal(out=inv[:], in_=inv[:])
                nrm = sb.tile([P, HC], f32, tag="nrm")
                nc.vector.tensor_mul(
                    out=nrm[:].rearrange("p (h c) -> p h c", h=H),
                    in0=pa[:, 0:HC].rearrange("p (h c) -> p h c", h=H),
                    in1=inv[:, :, None].to_broadcast([P, H, C]))
                m = sb.tile([P, C], f32, tag="mean")
                nc.vector.tensor_reduce(
                    out=m[:], in_=nrm[:].rearrange("p (h c) -> p c h", h=H),
                    axis=mybir.AxisListType.X, op=OP.add)
                nc.vector.scalar_tensor_tensor(
                    out=m[:], in0=m[:], scalar=1.0 / H, in1=brep_sb[:, :C],
                    op0=OP.mult, op1=OP.add)
                xf = sb.tile([P, C], f32, tag="xf")
                nc.scalar.activation(xf[:], m[:], AF.Relu)
                pt = pssm.tile([C, P], f32, space="PSUM", tag="fl")
                nc.tensor.transpose(out=pt[:], in_=xf[:], identity=idf_sb[:])
                xts = sb.tile([C, P], f32, tag="xts")
                nc.scalar.activation(xts[:], pt[:], AF.Copy)
                nc.sync.dma_start(out=xT_loc_dram[:, b * P:(b + 1) * P],
                                  in_=xts[:])
                write_adloc(xts[:], wadn_sb, C, Hn, adn, b)

            def flush_3(b, pa):
                nvalid = NVALID_LAST if b == NBLK - 1 else P
                inv = sb.tile([P, 1], f32, tag="inv")
                nc.vector.tensor_scalar_add(out=inv[:], in0=pa[:, 32:33],
                                            scalar1=1e-16)
                nc.vector.reciprocal(out=inv[:], in_=inv[:])
                z = sb.tile([P, 32], f32, tag="zf")
                nc.vector.tensor_mul(out=z[:], in0=pa[:, 0:32],
                                     in1=inv[:, 0:1].to_broadcast([P, 32]))
                nc.vector.tensor_add(out=z[:], in0=z[:], in1=b3r_sb[:])
                nc.sync.dma_start(out=z_out[b * P:b * P + nvalid, :],
                                  in_=z[:nvalid, :])
                zt_ps = pssm.tile([32, P], f32, space="PSUM", tag="fl")
                nc.tensor.transpose(out=zt_ps[:], in_=z[:, :32],
                                    identity=idf_sb[:])
                zt = sb.tile([32, P], f32, tag="zt")
                nc.vector.tensor_copy(out=zt[:], in_=zt_ps[:])
                pm = pssm.tile([P, 32], f32, space="PSUM", tag="fl")
                nc.tensor.matmul(out=pm[:], lhsT=zt[:], rhs=wm_sb[:],
                                 start=True, stop=True)
                zm = sb.tile([P, 32], f32, tag="zm")
                nc.vector.tensor_add(out=zm[:], in0=pm[:], in1=bmr_sb[:])
                nc.sync.dma_start(out=zm_out[b * P:b * P + nvalid, :],
                                  in_=zm[:nvalid, :])
                pv = pssm.tile([P, 32], f32, space="PSUM", tag="fl")
                nc.tensor.matmul(out=pv[:], lhsT=zt[:], rhs=wv_sb[:],
                                 start=True, stop=True)
                zv = sb.tile([P, 32], f32, tag="zv")
                nc.vector.tensor_add(out=zv[:], in0=pv[:], in1=bvr_sb[:])
                nc.scalar.activation(zv[:], zv[:], AF.Exp)
                nc.vector.tensor_tensor(out=zv[:], in0=zv[:], in1=c100_sb[:],
                                        op=OP.min)
                nc.vector.tensor_tensor(out=zv[:], in0=zv[:], in1=c1em8_sb[:],
                                        op=OP.max)
                nc.sync.dma_start(out=zv_out[b * P:b * P + nvalid, :],
                                  in_=zv[:nvalid, :])

            # ================ the program ==============================
            nc.sync.dma_start(out=xt_sb[:], in_=xT1f[:, :])
            for b in range(NBLK):
                write_adloc(xloc1_sb[:, b * P:(b + 1) * P], wad1_sb, 128, 4,
                            ad1, b)
            edge_layer(128, w1c_sb, ad1, 4, 64,
                       lambda b, pa: flush_12(b, pa, 4, 64, b1r_sb, x2T_loc,
                                              wad2_sb, ad2, 4))
            nc.gpsimd.collective_compute(
                "AllGather", mybir.AluOpType.bypass,
                replica_groups=[list(range(NCORES))],
                ins=[x2T_loc[:]], outs=[x2T_full[:]])
            for c in range(NCORES):
                eng = nc.sync if c % 2 == 0 else nc.scalar
                eng.dma_start(out=xt_sb[0:64, c * NPAD:(c + 1) * NPAD],
                              in_=x2T_full[c])
            edge_layer(64, w2c_sb, ad2, 4, 64,
                       lambda b, pa: flush_12(b, pa, 4, 64, b2r_sb, x3T_loc,
                                              wad3_sb, ad3, 1))
            nc.gpsimd.collective_compute(
                "AllGather", mybir.AluOpType.bypass,
                replica_groups=[list(range(NCORES))],
                ins=[x3T_loc[:]], outs=[x3T_full[:]])
            for c in range(NCORES):
                eng = nc.sync if c % 2 == 0 else nc.scalar
                eng.dma_start(out=xt_sb[0:64, c * NPAD:(c + 1) * NPAD],
                              in_=x3T_full[c])
            edge_layer(64, w3c_sb, ad3, 1, 32, flush_3)

    if do_compile:
        nc.compile()
    return nc


def _make_in_maps(x, params, wrap_src, indh, indTh):
    x = np.asarray(x, dtype=np.float32)

    def comb(W, a_s):
        W = np.asarray(W, np.float32)
        a_s = np.asarray(a_s, np.float32)
        heads, c = a_s.shape
        Wr = W.reshape(W.shape[0], heads, c)
        was = np.einsum('ihc,hc->ih', Wr, a_s)
        return np.concatenate([W, was], axis=1).astype(np.float32)

    def wadf(W, a_d):
        W = np.asarray(W, np.float32)
        a_d = np.asarray(a_d, np.float32)
        heads, c = a_d.shape
        Wr = W.reshape(W.shape[0], heads, c)
        return np.einsum('ihc,hc->ih', Wr, a_d).astype(np.float32)

    xT1f = np.zeros((P, NTOT), dtype=np.float32)
    for c in range(NCORES):
        xs = x[c * NPC:(c + 1) * NPC]
        xT1f[:, c * NPAD:c * NPAD + NPC] = xs.T

    def rep(v, n=P):
        v = np.asarray(v, np.float32).reshape(1, -1)
        return np.repeat(v, n, axis=0).astype(np.float32)

    def pad128(w):
        out = np.zeros((128, w.shape[1]), np.float32)
        out[:w.shape[0]] = w
        return out

    common = dict(
        xT1f=xT1f,
        c100=np.full((P, 32), 100.0, dtype=np.float32),
        c1em8=np.full((P, 32), 1e-8, dtype=np.float32),
        identf=np.eye(P, dtype=np.float32),
        w1c=comb(params['W1'], params['as1']),
        w2c=pad128(comb(params['W2'], params['as2'])),
        w3c=pad128(comb(params['W3'], params['as3'])),
        wad1=wadf(params['W1'], params['ad1']),
        wad2=wadf(params['W2'], params['ad2']),
        wad3=wadf(params['W3'], params['ad3']),
        b1r=rep(params['b1']), b2r=rep(params['b2']), b3r=rep(params['b3']),
        bmr=rep(params['bm']), bvr=rep(params['bv']),
        wm=np.asarray(params['Wm'], np.float32),
        wv=np.asarray(params['Wv'], np.float32),
    )
    in_maps = []
    for c in range(NCORES):
        m = dict(common)
        m.update(iwsrc=wrap_src[c], indh=indh[c], indTh=indTh[c],
                 xloc1=xT1f[:, c * NPAD:(c + 1) * NPAD].copy())
        in_maps.append(m)
    return in_maps


# ------------------------------------------------------------------ driver
def kernel(x, edge_index, W1, as1, ad1, b1, W2, as2, ad2, b2,
           W3, as3, ad3, b3, Wm, bm, Wv, bv):
    global LAST_RESULT
    import os
    from concourse.bass_utils import run_bass_kernel_spmd

    T, off8, offT, wrap_src, indh, indTh = _preprocess(
        np.asarray(edge_index))
    params = dict(W1=W1, as1=as1, ad1=ad1, b1=b1, W2=W2, as2=as2, ad2=ad2,
                  b2=b2, W3=W3, as3=as3, ad3=ad3, b3=b3, Wm=Wm, bm=bm,
                  Wv=Wv, bv=bv)
    in_maps = _make_in_maps(x, params, wrap_src, indh, indTh)

    nc = _build(T, off8, offT)
    res = run_bass_kernel_spmd(
        nc, in_maps, core_ids=list(range(NCORES)),
        trace=os.environ.get("BASS_TRACE", "") not in ("", "0"))
    LAST_RESULT = res

    z = np.concatenate([res.results[c]["z"] for c in range(NCORES)], axis=0)
    zm = np.concatenate([res.results[c]["zmean"] for c in range(NCORES)],
                        axis=0)
    zv = np.concatenate([res.results[c]["zvar"] for c in range(NCORES)],
                        axis=0)
    return zm, zv, z
